# revision 1
# baseline (speedup 1.0000x reference)
"""Longformer-style blocked local+global attention on 8 Trainium2 NeuronCores.

Problem (hardcoded): B=2, S=4096, D=1024, H=16, DH=64, W=256 (block size =
one-sided window radius), G=64 global prefix tokens.

Sharding: batch x head-group. Core c handles batch b = c//4 and heads
[(c%4)*4, (c%4)*4+4). Everything for one (b, head-group) is independent, so
there are no collectives; the only cross-core interaction is the final
output-projection partial sum, which the host performs (4 partials per batch).

Per-core kernel layout strategy:
  - Host passes x[b] pre-transposed (xT = [D, S]) so all projection matmuls
    have their contraction dim (D) on SBUF partitions.
  - q, k are produced directly in transposed layout qT/kT = [head_dim, S]
    (stationary = weight slice, moving = xT).
  - v is produced in natural layout [S, DH] per head, with a 65th column of
    ones appended so the PV matmul accumulates the softmax denominator as
    row 64 of the output for free.
  - Scores are computed transposed (sT = [key_pos, query]) so that exp'd
    scores feed the PV matmul directly as the moving operand (key_pos is the
    contraction dim) -- no transposes anywhere in the kernel.
  - The two heads of a head-pair occupy partition rows 0:64 / 64:128 of
    qT/kT; their score matmuls use disjoint PE row groups and are emitted
    adjacently so they stream concurrently through the array, and both
    heads' scores for one strip chunk are packed into one [128, 512] PSUM
    bank so each exp covers both heads in a single activation op.
  - Softmax uses exp without max subtraction (scores are O(1) by
    construction: scale 1/8, unit-variance inputs, 0.02-scaled weights), and
    masked entries are simply never computed (block structure) or zeroed via
    precomputed 0/1 triangular mask tiles after exp.
  - All matmul inputs are bf16 (fp32 matmul runs at 1/4 rate on trn2); all
    accumulation is fp32 in PSUM; softmax denominator/normalization is fp32.
"""

import numpy as np
import ml_dtypes

import concourse.bacc as bacc
import concourse.bass as bass
import concourse.mybir as mybir
import concourse.tile as tile
from concourse.bass_utils import run_bass_kernel_spmd

BF16 = mybir.dt.bfloat16
F32 = mybir.dt.float32
NPBF = ml_dtypes.bfloat16

B, S, D = 2, 4096, 1024
H, DH = 16, 64
W = 256          # block size == window radius
G = 64           # global prefix tokens
NB = S // W      # 16 blocks
SCALE = 1.0 / 8.0  # 1/sqrt(DH)

N_CORES = 8
HEADS_PER_CORE = 4
ECOLS = HEADS_PER_CORE * DH   # 256 embedding columns per core

# mask stack indices (each [128, 512] left-aligned, see build_masks)
M_L1, M_R0, M_EGEN, M_EN1, M_GC, M_R0R1, M_L0L1 = range(7)

# module-level caches
_BUILT = {}
LAST_RESULTS = None


def build_masks():
    """[7, 128, 512] bf16 0/1 masks, left-aligned.

    Local-strip chunk c of query block n holds key rows kj of block n-1
    (c=0,1), n (c=2,3), n+1 (c=4,5). Triangle masks (r = row within chunk,
    q = query within block): c0: q<=r (only q<128 possible); c1: q<=128+r;
    c4: q>=r; c5: q>=128+r (only q>=128 possible).
    """
    r = np.arange(128)[:, None]
    q = np.arange(256)[None, :]
    L0 = (q <= r).astype(np.float32)          # use cols 0:128
    L1 = (q <= 128 + r).astype(np.float32)
    R0 = (q >= r).astype(np.float32)
    R1 = (q >= 128 + r).astype(np.float32)    # use cols 128:256
    L0g = L0 * (r >= G)                       # left-upper chunk w/ global cut
    Gc = np.broadcast_to((r >= G).astype(np.float32), (128, 256)).copy()

    L0h, L0gh, R1h = L0[:, 0:128], L0g[:, 0:128], R1[:, 128:256]

    def pad(*parts):
        m = np.concatenate(parts, axis=1)
        if m.shape[1] < 512:
            m = np.concatenate(
                [m, np.zeros((128, 512 - m.shape[1]), np.float32)], axis=1)
        return m

    m = np.stack([
        pad(L1),             # M_L1
        pad(R0),             # M_R0
        pad(L0h, R1h),       # M_EGEN  [c0|c5]
        pad(L0gh, R1h),      # M_EN1   [c0 w/ global cut|c5] (n==1)
        pad(Gc),             # M_GC    (n==0 c2)
        pad(R0, R1h),        # M_R0R1  (n==0 [c4|c5])
        pad(L0h, L1),        # M_L0L1  (n==15 [c0|c1])
    ]).astype(NPBF)
    return m


def _tiles_for_block(n):
    """Per-head score-tile packing for query block n.

    Each head's active strip chunks (+ the global-prefix scores) are packed
    into three [128, 512] PSUM banks. Returns a list of tile specs:
      parts: [(c, col0, width, q0)] -- c in 0..5 or 'glb'; the chunk's
             scores land at psum/et cols [col0, col0+width), covering query
             range [q0, q0+width)
      exps:  [(col0, col1, rows)] -- merged exp ranges
      mask:  (col0, col1, mask_idx) or None -- post-exp 0/1 multiply; the
             mask tile is read left-aligned (cols [0, col1-col0))
    Chunk c covers sequence tile s_tile = 2*(n-1)+c.
    """
    if n == 0:
        return [
            dict(parts=[(2, 0, 256, 0), (3, 256, 256, 0)],
                 exps=[(0, 512, 128)], mask=(0, 256, M_GC)),
            dict(parts=[(4, 0, 256, 0), (5, 256, 128, 128)],
                 exps=[(0, 384, 128)], mask=(0, 384, M_R0R1)),
            dict(parts=[("glb", 0, 256, 0)], exps=[(0, 256, 64)], mask=None),
        ]
    if n == NB - 1:
        return [
            dict(parts=[(0, 0, 128, 0), (1, 128, 256, 0)],
                 exps=[(0, 384, 128)], mask=(0, 384, M_L0L1)),
            dict(parts=[(2, 0, 256, 0), (3, 256, 256, 0)],
                 exps=[(0, 512, 128)], mask=None),
            dict(parts=[("glb", 0, 256, 0)], exps=[(0, 256, 64)], mask=None),
        ]
    return [
        dict(parts=[(1, 0, 256, 0), (2, 256, 256, 0)],
             exps=[(0, 512, 128)], mask=(0, 256, M_L1)),
        dict(parts=[(3, 0, 256, 0), (4, 256, 256, 0)],
             exps=[(0, 512, 128)], mask=(256, 512, M_R0)),
        dict(parts=[(0, 0, 128, 0), (5, 128, 128, 128), ("glb", 256, 256, 0)],
             exps=[(0, 256, 128), (256, 512, 64)],
             mask=(0, 256, M_EN1 if n == 1 else M_EGEN)),
    ]


def build():
    """Build the per-core Bass/Tile program (identical on all 8 cores)."""
    nc = bacc.Bacc("TRN2", target_bir_lowering=False, debug=False)

    xT = nc.dram_tensor("xT", [D, S], BF16, kind="ExternalInput")
    wq = nc.dram_tensor("wq", [D, ECOLS], BF16, kind="ExternalInput")
    wk = nc.dram_tensor("wk", [D, ECOLS], BF16, kind="ExternalInput")
    wv = nc.dram_tensor("wv", [D, ECOLS], BF16, kind="ExternalInput")
    wo = nc.dram_tensor("wo", [ECOLS, D], BF16, kind="ExternalInput")
    masks = nc.dram_tensor("masks", [7, 128, 512], BF16, kind="ExternalInput")
    y = nc.dram_tensor("y", [S, D], F32, kind="ExternalOutput")

    EXP = mybir.ActivationFunctionType.Exp

    with tile.TileContext(nc) as tc:
        with (
            tc.tile_pool(name="const", bufs=1) as constp,
            tc.tile_pool(name="persist", bufs=1) as pers,
            tc.tile_pool(name="etp", bufs=4) as etp,
            tc.tile_pool(name="attnp", bufs=4) as atp,
            tc.tile_pool(name="smallp", bufs=8) as smp,
            tc.tile_pool(name="yp", bufs=2) as yp,
        ):
            # ---- constants ----
            wq_sb = constp.tile([128, 8, ECOLS], BF16, name="wq_sb")
            wk_sb = constp.tile([128, 8, ECOLS], BF16, name="wk_sb")
            wv_sb = constp.tile([128, 8, ECOLS], BF16, name="wv_sb")
            wo_sb = constp.tile([128, 2, D], BF16, name="wo_sb")
            mk_sb = constp.tile([128, 7, 512], BF16, name="mk_sb")
            for k in range(8):  # per-k-tile loads so the first matmuls start
                # early; gpsimd queue so they run parallel to sync's xT loads
                nc.gpsimd.dma_start(
                    out=wq_sb[:, k, :],
                    in_=wq.ap()[k * 128:(k + 1) * 128, :])
                nc.gpsimd.dma_start(
                    out=wk_sb[:, k, :],
                    in_=wk.ap()[k * 128:(k + 1) * 128, :])
                nc.gpsimd.dma_start(
                    out=wv_sb[:, k, :],
                    in_=wv.ap()[k * 128:(k + 1) * 128, :])
            nc.gpsimd.dma_start(
                out=wo_sb[:], in_=wo.ap().rearrange("(e p) d -> p e d", p=128))
            nc.gpsimd.dma_start(
                out=mk_sb[:], in_=masks.ap().rearrange("m p q -> p m q"))

            # ---- persistent per-head tensors ----
            qT = [pers.tile([128, S], BF16, name=f"qT{i}") for i in range(2)]
            kT = [pers.tile([128, S], BF16, name=f"kT{i}") for i in range(2)]
            # v natural layout: [128 seq-part, 32 seq-tiles, 4 heads, 65]
            # (col 64 = ones for the denominator row)
            vv = pers.tile([128, S // 128, HEADS_PER_CORE, 65], BF16, name="vv")
            for h in range(HEADS_PER_CORE):
                nc.vector.memset(vv[:, :, h, 64:65], 1.0)

            # ---- phase 1: projections ----
            with (
                tc.tile_pool(name="xstream", bufs=2) as xp,
                tc.tile_pool(name="ps_a", bufs=4, space="PSUM") as ps_a,
            ):
                for c in range(8):  # 512-column chunks of the sequence
                    xt = xp.tile([128, 8, 512], BF16, name="xt")
                    nc.sync.dma_start(
                        out=xt[:],
                        in_=xT.ap()[:, c * 512:(c + 1) * 512]
                            .rearrange("(k p) s -> p k s", p=128))
                    for hp in range(2):
                        for wsb, dst in ((wq_sb, qT[hp]), (wk_sb, kT[hp])):
                            ps = ps_a.tile([128, 512], F32, name="ps_qk",
                                           tag="psa")
                            for k in range(8):
                                nc.tensor.matmul(
                                    ps[:],
                                    wsb[:, k, hp * 128:(hp + 1) * 128],
                                    xt[:, k, :],
                                    start=(k == 0), stop=(k == 7))
                            nc.vector.tensor_copy(
                                dst[:, c * 512:(c + 1) * 512], ps[:])
                    for ss in range(4):  # 128-row seq subtiles -> v natural
                        ps = ps_a.tile([128, ECOLS], F32, name="ps_v",
                                       tag="psa")
                        for k in range(8):
                            nc.tensor.matmul(
                                ps[:],
                                xt[:, k, ss * 128:(ss + 1) * 128],
                                wv_sb[:, k, :],
                                start=(k == 0), stop=(k == 7))
                        nc.vector.tensor_copy(
                            vv[:, c * 4 + ss, :, 0:64],
                            ps[:].rearrange("p (h e) -> p h e", h=4))

            # ---- phase 2: attention + output projection ----
            with (
                tc.tile_pool(name="ps_s", bufs=5, space="PSUM") as ps_sp,
                tc.tile_pool(name="ps_o", bufs=1, space="PSUM") as ps_op,
                tc.tile_pool(name="ps_y", bufs=2, space="PSUM") as ps_yp,
            ):
                def emit_wo(n, at_blk):
                    """Output projection for block n's 256 rows."""
                    for ss in range(2):
                        ysb = yp.tile([128, D], F32, name="ysb")
                        for dk in range(2):
                            py_ = ps_yp.tile([128, 512], F32, name="py")
                            for e in range(2):
                                nc.tensor.matmul(
                                    py_[:],
                                    at_blk[:, e, ss * 128:(ss + 1) * 128],
                                    wo_sb[:, e, dk * 512:(dk + 1) * 512],
                                    start=(e == 0), stop=(e == 1))
                            if dk == 0:
                                nc.vector.tensor_copy(
                                    ysb[:, 0:512], py_[:])
                            else:
                                nc.scalar.copy(ysb[:, 512:1024], py_[:])
                        r0 = n * 256 + ss * 128
                        nc.sync.dma_start(out=y.ap()[r0:r0 + 128, :],
                                          in_=ysb[:])

                # Wo for block n is emitted after block n+1's attention so
                # the in-order PE never stalls on the normalize chain.
                pending = None
                for n in range(NB):
                    at_blk = atp.tile([128, 2, 256], BF16, name="at_blk")
                    tiles = _tiles_for_block(n)
                    for hp in range(2):
                        qpair = qT[hp]
                        kpair = kT[hp]
                        # et: [128, slot, 512]; slot = tile_idx*2 + hh; the
                        # col layout inside a slot mirrors the psum packing
                        et = etp.tile([128, 6, 512], BF16, name="et")
                        loc = {}  # c -> (tile_idx, col0, width, q0)
                        for ti, sp in enumerate(tiles):
                            for hh in range(2):
                                hr = hh * 64
                                slot = ti * 2 + hh
                                st = ps_sp.tile([128, 512], F32, name="st",
                                                tag="st")
                                for c, col0, width, q0 in sp["parts"]:
                                    if c == "glb":
                                        lhs = kpair[hr:hr + 64, 0:G]
                                        rows = 64
                                    else:
                                        s0 = (2 * (n - 1) + c) * 128
                                        lhs = kpair[hr:hr + 64, s0:s0 + 128]
                                        rows = 128
                                    nc.tensor.matmul(
                                        st[0:rows, col0:col0 + width],
                                        lhs,
                                        qpair[hr:hr + 64,
                                              n * 256 + q0:
                                              n * 256 + q0 + width],
                                        start=True, stop=True)
                                    loc[c] = (ti, col0, width, q0)
                                for c0e, c1e, rows in sp["exps"]:
                                    nc.scalar.activation(
                                        et[0:rows, slot, c0e:c1e],
                                        st[0:rows, c0e:c1e], EXP, scale=SCALE)
                                if sp["mask"] is not None:
                                    m0, m1, mi = sp["mask"]
                                    nc.vector.tensor_mul(
                                        et[:, slot, m0:m1],
                                        et[:, slot, m0:m1],
                                        mk_sb[:, mi, 0:m1 - m0])

                        # PV + normalize; the two heads share one psum bank
                        pv_order = [c for c in (3, 2, 1, 4, 0, 5) if c in loc]
                        ot = ps_op.tile([128, 512], F32, name="ot")
                        for hh in range(2):
                            h = hp * 2 + hh
                            ob = hh * 256
                            for i, c in enumerate(pv_order):
                                ti, col0, width, q0 = loc[c]
                                s_tile = 2 * (n - 1) + c
                                nc.tensor.matmul(
                                    ot[0:65, ob + q0:ob + q0 + width],
                                    vv[:, s_tile, h, :],
                                    et[:, ti * 2 + hh, col0:col0 + width],
                                    start=(i == 0), stop=False)
                            ti, col0, width, q0 = loc["glb"]
                            nc.tensor.matmul(
                                ot[0:65, ob:ob + 256],
                                vv[0:64, 0, h, :],
                                et[0:64, ti * 2 + hh, col0:col0 + width],
                                start=False, stop=True)
                        for hh in range(2):
                            ob = hh * 256
                            # reciprocal_approx_fast needs exact fp32 bits;
                            # its PSUM read path perturbs them (HW-measured
                            # ~5% error), so bounce the row through SBUF.
                            row = smp.tile([1, 256], F32, name="row")
                            nc.vector.tensor_copy(row[:],
                                                  ot[64:65, ob:ob + 256])
                            den = smp.tile([1, 256], F32, name="den")
                            nc.vector.reciprocal_approx_fast(den[:], row[:])
                            recb = smp.tile([64, 256], F32, name="recb")
                            nc.gpsimd.partition_broadcast(recb[:], den[:])
                            nc.vector.tensor_mul(
                                at_blk[hh * 64:(hh + 1) * 64, hp, :],
                                ot[0:64, ob:ob + 256], recb[:])

                    if pending is not None:
                        emit_wo(*pending)
                    pending = (n, at_blk)
                emit_wo(*pending)

    nc.compile()
    return nc


def _get_nc():
    if "nc" not in _BUILT:
        _BUILT["nc"] = build()
    return _BUILT["nc"]


def make_in_maps(x, Wq, Wk, Wv, Wo):
    masks_np = build_masks()
    xT = [np.ascontiguousarray(x[b].T).astype(NPBF) for b in range(B)]
    wq16, wk16, wv16 = (w.astype(NPBF) for w in (Wq, Wk, Wv))
    wo16 = Wo.astype(NPBF)
    in_maps = []
    for core in range(N_CORES):
        b, hg = core // 4, core % 4
        cols = slice(hg * ECOLS, (hg + 1) * ECOLS)
        in_maps.append({
            "xT": xT[b],
            "wq": np.ascontiguousarray(wq16[:, cols]),
            "wk": np.ascontiguousarray(wk16[:, cols]),
            "wv": np.ascontiguousarray(wv16[:, cols]),
            "wo": np.ascontiguousarray(wo16[cols, :]),
            "masks": masks_np,
        })
    return in_maps


def kernel(x, Wq, Wk, Wv, Wo):
    global LAST_RESULTS
    nc = _get_nc()
    in_maps = make_in_maps(x, Wq, Wk, Wv, Wo)
    res = run_bass_kernel_spmd(nc, in_maps, core_ids=list(range(N_CORES)))
    LAST_RESULTS = res
    out = np.zeros((B, S, D), np.float32)
    for core in range(N_CORES):
        out[core // 4] += res.results[core]["y"]
    return out

